# revision 18
# baseline (speedup 1.0000x reference)
"""Trainium2 Bass kernel for the CTM token2map/conv3x3s2/map2token/BN/cluster module.

8 cores; core c processes batch c % 4 end-to-end (cores 4-7 compute the same
batches redundantly; the host reads outputs from cores 0-3). Only cross-core
exchange: BatchNorm statistics AllReduce over groups [[0,1,2,3],[4,5,6,7]].

Static shapes: B=4, N=16384, C_in=128, C_out=256, H=W=256, N0=65536,
Hd=Wd=128, Hc=Wc=64.

Scatter-add strategy: the HW dma_scatter_add races on duplicate indices, so
each 128-token scatter call first combines within-chunk duplicates with a
selection-matrix matmul (S @ payload), keeps only the first occurrence per key
(others redirected to a dump row), and consecutive calls serialize through
Tile's WAW dependency on the table. Map rows exceed int16, so the map is kept
as two tables of cell-quads (even/odd cell pairs) with dump-row masking.
"""
import sys
sys.path.insert(0, "/opt/trn_rl_repo")
sys.path.insert(0, "/opt/trn_rl_repo/concourse")

import dataclasses
import os
import numpy as np

import concourse.bass as bass
import concourse.bacc as bacc
import concourse.bass_isa as bass_isa
import concourse.tile as tile
import concourse.mybir as mybir
from concourse import bass_utils
from concourse.masks import make_identity

F32 = mybir.dt.float32
BF16 = mybir.dt.bfloat16
I32 = mybir.dt.int32
I16 = mybir.dt.int16
ALU = mybir.AluOpType
ACTF = mybir.ActivationFunctionType
AX = mybir.AxisListType

N = 16384
CIN = 128
COUT = 256
H = 256
N0 = 65536
HD = 128
NC64 = 4096
P = 128
NCOL = N0 // P          # 512
EPS = 1e-6
BN_EPS = 1e-5
TW = 320
DUMP_A = 16384
DUMP_C = 16384
DUMP_E = 4096
BIG = 16384.0

_CACHE = {}
PHASES = int(os.environ.get("CTM_PHASES", "9"))
DEBUG = int(os.environ.get("CTM_DEBUG", "0"))


def _body(nc, tc, T):
    x_t, loc_t, ia_t, aw_t = T["x_t"], T["loc_t"], T["ia_t"], T["aw_t"]
    cw_t, cb_t, skw_t = T["cw_t"], T["cb_t"], T["skw_t"]
    bng_t, bnb_t, cfw_t, cfb_t = T["bng_t"], T["bnb_t"], T["cfw_t"], T["cfb_t"]
    xdown_t, xout_t, conf_t, awd_t, iad_t = (
        T["xdown_t"], T["xout_t"], T["conf_t"], T["awd_t"], T["iad_t"])

    dram = tc.alloc_tile_pool(name="dram", bufs=1, space="DRAM")
    cpool = tc.alloc_tile_pool(name="consts", bufs=1)
    psum = tc.alloc_tile_pool(name="psum", bufs=2, space="PSUM")
    psum1 = tc.alloc_tile_pool(name="psum1", bufs=1, space="PSUM")
    persist = tc.alloc_tile_pool(name="persist", bufs=1)
    bulk = tc.alloc_tile_pool(name="bulk", bufs=1)
    btmp = tc.alloc_tile_pool(name="btmp", bufs=1)

    # DRAM scratch
    mapA = dram.tile([DUMP_A + 64, TW], F32)
    mapB = dram.tile([DUMP_A + 64, TW], F32)
    y_tbl = dram.tile([HD * HD + 256, COUT], BF16)
    tok_tbl = dram.tile([DUMP_C + 64, TW], F32)
    tokn_tbl = dram.tile([N, 384], BF16)
    clu_tbl = dram.tile([DUMP_E + 64, TW], F32)
    rec64_tbl = dram.tile([NC64, 64], F32)
    idx_dram = dram.tile([8, P, NCOL * 8], I16)   # staged wrap-idx tensors
    x_int = dram.tile([N, CIN], F32)
    bncA = dram.tile([1, 2 * COUT], F32)
    bncB = dram.tile([1, 2 * COUT], F32)

    # ---------------- constants ----------------
    ident = cpool.tile([P, P], F32)
    make_identity(nc, ident[:])
    identb = cpool.tile([P, P], BF16)
    nc.vector.tensor_copy(identb[:], ident[:])
    ones_col = cpool.tile([P, 1], F32)
    nc.vector.memset(ones_col[:], 1.0)
    posrow = cpool.tile([P, P], F32)
    nc.gpsimd.iota(posrow[:], pattern=[[1, P]], base=0, channel_multiplier=0,
                   allow_small_or_imprecise_dtypes=True)
    poscol = cpool.tile([P, 1], F32)
    nc.gpsimd.iota(poscol[:], pattern=[[1, 1]], base=0, channel_multiplier=1,
                   allow_small_or_imprecise_dtypes=True)

    def wrap16(src_f32, slot):
        """[128, NCOL] f32 -> int16 16-wrap [128, NCOL*8] staged to idx_dram."""
        w = btmp.tile([P, NCOL * 8], I16, tag="w16tmp")
        wv = w[0:16, :].rearrange("p (c q) -> p c q", q=8)
        for q in range(8):
            pt = psum.tile([16, NCOL], F32, tag="wrapp")
            nc.tensor.matmul(pt[:], lhsT=ident[:, 16 * q:16 * (q + 1)],
                             rhs=src_f32, start=True, stop=True)
            nc.vector.tensor_copy(wv[:, :, q], pt[:])
        for r in range(1, 8):
            nc.sync.dma_start(w[16 * r:16 * (r + 1), :], w[0:16, :])
        nc.sync.dma_start(idx_dram[slot, :, :], w[:])

    # stage x into internal DRAM (gather sources must be internal)
    for r0 in range(0, N, P):
        st = btmp.tile([P, CIN], F32, tag="xstage")
        nc.sync.dma_start(st[:], x_t.ap()[r0:r0 + P, :])
        nc.sync.dma_start(x_int[r0:r0 + P, :], st[:])

    # ---------------- per-point bulk math ----------------
    locv = btmp.tile([P, NCOL, 2], F32)
    nc.sync.dma_start(locv[:], loc_t.ap().rearrange("(p c) d -> p c d", p=P))
    iag = btmp.tile([P, NCOL], I32)
    nc.sync.dma_start(iag[:], ia_t.ap().rearrange("(p c) -> p c", p=P))
    aggw = bulk.tile([P, NCOL], F32)
    nc.sync.dma_start(aggw[:], aw_t.ap().rearrange("(p c) -> p c", p=P))

    lx = locv[:, :, 0]
    ly = locv[:, :, 1]

    def grid_i32(out_i32, l, mul, add, hi):
        t = btmp.tile([P, NCOL], F32, tag="gtmp")
        nc.vector.tensor_scalar(t[:], l, float(mul), float(add),
                                op0=ALU.mult, op1=ALU.add)
        nc.vector.tensor_scalar(t[:], t[:], 0.0, float(hi),
                                op0=ALU.max, op1=ALU.min)
        nc.vector.tensor_copy(out_i32[:], t[:])   # f32->i32 cast: half-to-even

    px = btmp.tile([P, NCOL], I32, tag="px")
    py = btmp.tile([P, NCOL], I32, tag="py")
    grid_i32(px, lx, 128.0, 127.5, 255.0)
    grid_i32(py, ly, 128.0, 127.5, 255.0)
    cell = btmp.tile([P, NCOL], I32)
    nc.vector.tensor_scalar(cell[:], py[:], 8, None, op0=ALU.logical_shift_left)
    nc.vector.tensor_tensor(cell[:], cell[:], px[:], op=ALU.add)

    grid_i32(px, lx, 32.0, 31.5, 63.0)
    grid_i32(py, ly, 32.0, 31.5, 63.0)
    cell64 = btmp.tile([P, NCOL], I32)
    nc.vector.tensor_scalar(cell64[:], py[:], 6, None, op0=ALU.logical_shift_left)
    nc.vector.tensor_tensor(cell64[:], cell64[:], px[:], op=ALU.add)
    nc.sync.dma_start(iad_t.ap().rearrange("(p c) -> p c", p=P), cell64[:])

    cell64f = bulk.tile([P, NCOL], F32)
    nc.vector.tensor_copy(cell64f[:], cell64[:])
    iagf = bulk.tile([P, NCOL], F32)
    nc.vector.tensor_copy(iagf[:], iag[:])

    pairf = bulk.tile([P, NCOL], F32)
    quadf = btmp.tile([P, NCOL], F32)
    parf = bulk.tile([P, NCOL], F32)
    notparf = bulk.tile([P, NCOL], F32)
    pbit = btmp.tile([P, NCOL], F32)
    notpbit = btmp.tile([P, NCOL], F32)
    ti = btmp.tile([P, NCOL], I32, tag="ti")
    nc.vector.tensor_scalar(ti[:], cell[:], 1, None, op0=ALU.logical_shift_right)
    nc.vector.tensor_copy(pairf[:], ti[:])
    nc.vector.tensor_scalar(ti[:], cell[:], 2, None, op0=ALU.logical_shift_right)
    nc.vector.tensor_copy(quadf[:], ti[:])
    nc.vector.tensor_scalar(ti[:], cell[:], 1, None, op0=ALU.bitwise_and)
    nc.vector.tensor_copy(parf[:], ti[:])
    nc.vector.tensor_scalar(notparf[:], parf[:], 1.0, -1.0,
                            op0=ALU.subtract, op1=ALU.mult)
    nc.vector.tensor_scalar(ti[:], cell[:], 2, None, op0=ALU.bitwise_and)
    nc.vector.tensor_copy(pbit[:], ti[:])
    nc.vector.tensor_scalar(pbit[:], pbit[:], 0.5, None, op0=ALU.mult)
    nc.vector.tensor_scalar(notpbit[:], pbit[:], 1.0, -1.0,
                            op0=ALU.subtract, op1=ALU.mult)

    # bilinear coords
    fx = btmp.tile([P, NCOL], F32)
    fy = btmp.tile([P, NCOL], F32)
    nc.vector.tensor_scalar(fx[:], lx, 64.0, 63.5, op0=ALU.mult, op1=ALU.add)
    nc.vector.tensor_scalar(fx[:], fx[:], 0.0, 127.0, op0=ALU.max, op1=ALU.min)
    nc.vector.tensor_scalar(fy[:], ly, 64.0, 63.5, op0=ALU.mult, op1=ALU.add)
    nc.vector.tensor_scalar(fy[:], fy[:], 0.0, 127.0, op0=ALU.max, op1=ALU.min)

    def floor_clamped(out, f):
        ii = btmp.tile([P, NCOL], I32, tag="fltmp")
        ff = btmp.tile([P, NCOL], F32, tag="fltmp2")
        nc.vector.tensor_copy(ii[:], f)
        nc.vector.tensor_copy(ff[:], ii[:])
        gt = btmp.tile([P, NCOL], F32, tag="fltmp3")
        nc.vector.tensor_tensor(gt[:], ff[:], f, op=ALU.is_gt)
        nc.vector.tensor_tensor(out[:], ff[:], gt[:], op=ALU.subtract)
        nc.vector.tensor_scalar(out[:], out[:], 126.0, None, op0=ALU.min)

    x0 = btmp.tile([P, NCOL], F32)
    y0 = btmp.tile([P, NCOL], F32)
    floor_clamped(x0, fx[:])
    floor_clamped(y0, fy[:])
    wxv = btmp.tile([P, NCOL], F32)
    wyv = btmp.tile([P, NCOL], F32)
    nc.vector.tensor_tensor(wxv[:], fx[:], x0[:], op=ALU.subtract)
    nc.vector.tensor_tensor(wyv[:], fy[:], y0[:], op=ALU.subtract)
    g1i = btmp.tile([P, NCOL], F32)
    nc.vector.tensor_scalar(g1i[:], y0[:], 128.0, None, op0=ALU.mult)
    nc.vector.tensor_tensor(g1i[:], g1i[:], x0[:], op=ALU.add)
    g2i = btmp.tile([P, NCOL], F32)
    nc.vector.tensor_scalar(g2i[:], g1i[:], 128.0, None, op0=ALU.add)

    wxb = btmp.tile([P, NCOL], F32)
    wyb = btmp.tile([P, NCOL], F32)
    nc.vector.tensor_scalar(wxb[:], wxv[:], 1.0, -1.0, op0=ALU.subtract, op1=ALU.mult)
    nc.vector.tensor_scalar(wyb[:], wyv[:], 1.0, -1.0, op0=ALU.subtract, op1=ALU.mult)
    wq = {}
    for name, a, b in (("w00", wxb, wyb), ("w01", wxv, wyb),
                       ("w10", wxb, wyv), ("w11", wxv, wyv)):
        w = btmp.tile([P, NCOL], F32, tag=name)
        nc.vector.tensor_tensor(w[:], a[:], b[:], op=ALU.mult)
        nc.vector.tensor_tensor(w[:], w[:], aggw[:], op=ALU.mult)
        wb = bulk.tile([P, NCOL], BF16, tag=name + "b")
        nc.vector.tensor_copy(wb[:], w[:])
        wq[name] = wb

    # ---------------- first-occurrence masks ----------------
    def first_mask(keyf, out_first):
        pr = posrow[:]
        posr3 = dataclasses.replace(pr, ap=[pr.ap[0], [0, 4], pr.ap[1]])
        for c0 in range(0, NCOL, 4):
            tp = psum.tile([P, 4 * P], F32, tag="wrapp")
            for k in range(4):
                nc.tensor.transpose(
                    tp[:, k * P:(k + 1) * P],
                    in_=keyf[:, c0 + k:c0 + k + 1].to_broadcast([P, P]),
                    identity=ident[:])
            tw_ = btmp.tile([P, 4, P], F32, tag="fmw")
            nc.vector.tensor_copy(tw_[:], tp[:].rearrange("p (a x) -> p a x", a=4))
            s = btmp.tile([P, 4, P], F32, tag="fms")
            ks = keyf[:, c0:c0 + 4]
            ks3 = dataclasses.replace(ks, ap=[ks.ap[0], ks.ap[1], [0, P]])
            nc.vector.tensor_tensor(s[:], ks3, tw_[:], op=ALU.is_equal)
            m = btmp.tile([P, 4, P], F32, tag="fmm")
            nc.vector.tensor_scalar(m[:], s[:], -BIG, None, op0=ALU.mult)
            nc.vector.tensor_tensor(m[:], m[:], posr3, op=ALU.add)
            mn = btmp.tile([P, 4], F32, tag="fmn")
            nc.vector.tensor_reduce(mn[:], m[:], axis=AX.X, op=ALU.min)
            nc.vector.tensor_scalar(mn[:], mn[:], BIG, None, op0=ALU.add)
            nc.vector.tensor_tensor(out_first[:, c0:c0 + 4], mn[:],
                                    poscol[:].to_broadcast([P, 4]), op=ALU.is_equal)

    firstA = btmp.tile([P, NCOL], F32)
    first_mask(pairf[:], firstA)
    firstC = btmp.tile([P, NCOL], F32)
    first_mask(iagf[:], firstC)
    firstE = btmp.tile([P, NCOL], F32)
    first_mask(cell64f[:], firstE)

    def dump_select(keyf, mask, dump, tag):
        o = btmp.tile([P, NCOL], F32, tag=tag)
        nc.vector.tensor_scalar(o[:], keyf, float(dump), None, op0=ALU.subtract)
        nc.vector.tensor_tensor(o[:], o[:], mask, op=ALU.mult)
        nc.vector.tensor_scalar(o[:], o[:], float(dump), None, op0=ALU.add)
        return o

    mA = btmp.tile([P, NCOL], F32, tag="mA")
    nc.vector.tensor_tensor(mA[:], firstA[:], notpbit[:], op=ALU.mult)
    mB = btmp.tile([P, NCOL], F32, tag="mB")
    nc.vector.tensor_tensor(mB[:], firstA[:], pbit[:], op=ALU.mult)
    idxA_f = dump_select(quadf[:], mA[:], DUMP_A, "idxAf")
    idxB_f = dump_select(quadf[:], mB[:], DUMP_A, "idxBf")
    idxC_f = dump_select(iagf[:], firstC[:], DUMP_C, "idxCf")
    idxE_f = dump_select(cell64f[:], firstE[:], DUMP_E, "idxEf")

    if DEBUG:
        dbg2 = btmp.tile([P, 4 * NCOL], F32, tag="dbg2")
        nc.vector.tensor_copy(dbg2[:, 0:NCOL], pairf[:])
        nc.vector.tensor_copy(dbg2[:, NCOL:2 * NCOL], firstA[:])
        nc.vector.tensor_copy(dbg2[:, 2 * NCOL:3 * NCOL], idxA_f[:])
        nc.vector.tensor_copy(dbg2[:, 3 * NCOL:4 * NCOL], idxB_f[:])
        nc.sync.dma_start(T["dbg2_t"].ap(), dbg2[:])

    # slots: 0=x/idx_agg 1=A 2=B 3=C 4=E 5=g1 6=g2 7=cell64(no dump)
    wrap16(iagf[:], 0)
    wrap16(idxA_f[:], 1)
    wrap16(idxB_f[:], 2)
    wrap16(idxC_f[:], 3)
    wrap16(idxE_f[:], 4)
    wrap16(g1i[:], 5)
    wrap16(g2i[:], 6)
    wrap16(cell64f[:], 7)

    btmp.release()

    def load_idx(pool, slot):
        t = pool.tile([P, NCOL * 8], I16, tag="idx%d" % slot)
        nc.sync.dma_start(t[:], idx_dram[slot, :, :])
        return t

    if DEBUG:
        with tc.tile_pool(name="dbgi", bufs=1) as dbgi:
            tA = dbgi.tile([P, NCOL * 8], I16, tag="tA")
            nc.sync.dma_start(tA[:], idx_dram[1, :, :])
            nc.sync.dma_start(T["dbg3_t"].ap()[:, 0:TW], tA[:, 0:TW])
            tB = dbgi.tile([P, NCOL * 8], I16, tag="tB")
            nc.sync.dma_start(tB[:], idx_dram[2, :, :])
            nc.sync.dma_start(T["dbg3_t"].ap()[:, TW:2 * TW], tB[:, 0:TW])

    # zero tables
    zt = cpool.tile([P, TW], F32)
    nc.vector.memset(zt[:], 0.0)
    for tbl, nrows in ((mapA, DUMP_A + 64), (mapB, DUMP_A + 64),
                       (tok_tbl, DUMP_C + 64), (clu_tbl, DUMP_E + 64)):
        for r0 in range(0, nrows, P):
            n_ = min(P, nrows - r0)
            nc.sync.dma_start(tbl[r0:r0 + n_, :], zt[:n_, :])

    # ---------------- phase A ----------------
    PIECE_A = 4
    with tc.tile_pool(name="apool", bufs=2) as apool, \
         tc.tile_pool(name="achain", bufs=4) as chain, \
         tc.tile_pool(name="aidx", bufs=1) as aidx:
        i16x = load_idx(aidx, 0)
        i16A = load_idx(aidx, 1)
        i16B = load_idx(aidx, 2)
        for piece in range(NCOL // PIECE_A):
            g = apool.tile([P, PIECE_A, CIN], F32, tag="agbuf")
            nc.gpsimd.dma_gather(
                g[:], x_int[:], i16x[:, piece * PIECE_A * 8:(piece + 1) * PIECE_A * 8],
                P * PIECE_A, P * PIECE_A, CIN)
            for j in range(PIECE_A):
                c = piece * PIECE_A + j
                pay = chain.tile([P, TW], F32, tag="apay")
                nc.vector.memset(pay[:, 129:160], 0.0)
                nc.vector.memset(pay[:, 289:320], 0.0)
                nc.vector.tensor_tensor(
                    pay[:, 0:CIN], g[:, j, :],
                    notparf[:, c:c + 1].to_broadcast([P, CIN]), op=ALU.mult)
                nc.vector.tensor_copy(pay[:, 128:129], notparf[:, c:c + 1])
                nc.vector.tensor_tensor(
                    pay[:, 160:160 + CIN], g[:, j, :],
                    parf[:, c:c + 1].to_broadcast([P, CIN]), op=ALU.mult)
                nc.vector.tensor_copy(pay[:, 288:289], parf[:, c:c + 1])
                tp = psum.tile([P, P], F32, tag="tpp")
                nc.tensor.transpose(
                    tp[:], in_=pairf[:, c:c + 1].to_broadcast([P, P]),
                    identity=ident[:])
                tw_ = chain.tile([P, P], F32, tag="atw")
                nc.vector.tensor_copy(tw_[:], tp[:])
                s = chain.tile([P, P], F32, tag="asel")
                nc.vector.tensor_tensor(
                    s[:], pairf[:, c:c + 1].to_broadcast([P, P]), tw_[:],
                    op=ALU.is_equal)
                cp = psum.tile([P, TW], F32, tag="cpp")
                nc.tensor.matmul(cp[:], lhsT=s[:], rhs=pay[:], start=True, stop=True)
                paysc = chain.tile([P, 1, TW], F32, tag="apaysc")
                nc.scalar.copy(paysc[:, 0, :], cp[:])
                nc.gpsimd.dma_scatter_add(
                    mapA[:], paysc[:], i16A[:, c * 8:(c + 1) * 8],
                    P, P, TW, elem_step=TW)
                nc.gpsimd.dma_scatter_add(
                    mapB[:], paysc[:], i16B[:, c * 8:(c + 1) * 8],
                    P, P, TW, elem_step=TW)

    if DEBUG:
        with tc.tile_pool(name="dbgm", bufs=2) as dp:
            for t in range(16):
                r_ = dp.tile([P, TW], F32, tag="dbgm")
                nc.sync.dma_start(r_[:], mapA[t * P:(t + 1) * P, :])
                nc.sync.dma_start(T["dbgmap_t"].ap()[t * P:(t + 1) * P, :], r_[:])

    # ---------------- phase B ----------------
    if PHASES < 2:
        bulk.release(); persist.release(); psum1.release(); psum.release()
        cpool.release(); dram.release()
        return
    wk = cpool.tile([CIN, 9, COUT], BF16)
    cwf = cpool.tile([CIN, 9, COUT], F32)
    nc.sync.dma_start(cwf[:], cw_t.ap().rearrange("(k c) o -> c k o", c=CIN))
    nc.vector.tensor_copy(wk[:], cwf[:])
    brow = cpool.tile([P, COUT], F32)
    b1 = cpool.tile([1, COUT], F32)
    nc.sync.dma_start(b1[:], cb_t.ap())
    nc.gpsimd.partition_broadcast(brow[:], b1[:], channels=P)

    win = persist.tile([P, 4, 257], BF16)

    with tc.tile_pool(name="bpool", bufs=4) as bpool:
        def transpose_maprow(y):
            blk = bpool.tile([P, TW], F32, tag="bblk")
            nc.sync.dma_start(blk[0:P:2, :], mapA[64 * y:64 * (y + 1), :])
            nc.sync.dma_start(blk[1:P:2, :], mapB[64 * y:64 * (y + 1), :])
            slot = y % 4
            for half, off in ((0, 0), (1, 160)):
                rec = bpool.tile([P, 1], F32, tag="brec")
                nc.vector.tensor_scalar(rec[:], blk[:, 128 + off:129 + off], EPS,
                                        None, op0=ALU.add)
                nc.vector.reciprocal(rec[:], rec[:])
                dg = bpool.tile([P, P], BF16, tag="bdg")
                nc.vector.tensor_tensor(dg[:], ident[:],
                                        rec[:].to_broadcast([P, P]), op=ALU.mult)
                bb = bpool.tile([P, P], BF16, tag="bbb")
                nc.vector.tensor_copy(bb[:], blk[:, off:off + CIN])
                tp = psum.tile([P, P], F32, tag="tpp")
                nc.tensor.matmul(tp[:], lhsT=bb[:], rhs=dg[:], start=True, stop=True)
                if half == 0:
                    nc.vector.tensor_copy(win[:, slot, 0:P], tp[:])
                else:
                    nc.vector.tensor_copy(win[:, slot, P + 1:2 * P + 1], tp[:])
            nc.vector.memset(win[:, slot, P:P + 1], 0.0)

        transpose_maprow(0)
        transpose_maprow(1)
        for r in range(HD):
            if r > 0:
                transpose_maprow(2 * r)
                transpose_maprow(2 * r + 1)
            yp = psum.tile([P, COUT], F32, tag="cpp")
            started = False
            for ky in range(3):
                ry = 2 * r - 1 + ky
                if ry < 0 or ry >= H:
                    continue
                slot = ry % 4
                nc.tensor.matmul(
                    yp[:], lhsT=win[:, slot, 0:P],
                    rhs=wk[:, ky * 3 + 1, :],
                    start=not started, stop=False)
                started = True
                nc.tensor.matmul(
                    yp[:], lhsT=win[:, slot, P:2 * P],
                    rhs=wk[:, ky * 3 + 0, :],
                    start=False, stop=False)
                nc.tensor.matmul(
                    yp[:], lhsT=win[:, slot, P + 1:2 * P + 1],
                    rhs=wk[:, ky * 3 + 2, :],
                    start=False, stop=(ky == 2))
            yo = bpool.tile([P, COUT], BF16, tag="byo")
            nc.vector.tensor_tensor(yo[:], yp[:], brow[:], op=ALU.add)
            nc.sync.dma_start(y_tbl[r * P:(r + 1) * P, :], yo[:])

    if DEBUG:
        with tc.tile_pool(name="dbgy", bufs=2) as dp:
            for t in range(HD):
                r_ = dp.tile([P, COUT], BF16, tag="dbgyb")
                nc.sync.dma_start(r_[:], y_tbl[t * P:(t + 1) * P, :])
                rf = dp.tile([P, COUT], F32, tag="dbgyf")
                nc.vector.tensor_copy(rf[:], r_[:])
                nc.sync.dma_start(T["dbgy_t"].ap()[t * P:(t + 1) * P, :], rf[:])

    # ---------------- phase C ----------------
    if PHASES < 3:
        bulk.release(); persist.release(); psum1.release(); psum.release()
        cpool.release(); dram.release()
        return
    yv = dataclasses.replace(y_tbl[:], ap=[[COUT, HD * HD], [1, 2 * COUT]])
    PIECE_C = 4
    with tc.tile_pool(name="cpoolg", bufs=2) as cpoolg, \
         tc.tile_pool(name="cchain", bufs=4) as chain, \
         tc.tile_pool(name="cidx", bufs=1) as cidx:
        i16g1 = load_idx(cidx, 5)
        i16g2 = load_idx(cidx, 6)
        i16C = load_idx(cidx, 3)
        for piece in range(NCOL // PIECE_C):
            g1 = cpoolg.tile([P, PIECE_C, 2 * COUT], BF16, tag="cg1")
            g2 = cpoolg.tile([P, PIECE_C, 2 * COUT], BF16, tag="cg2")
            nc.gpsimd.dma_gather(
                g1[:], yv, i16g1[:, piece * PIECE_C * 8:(piece + 1) * PIECE_C * 8],
                P * PIECE_C, P * PIECE_C, 2 * COUT, elem_step=COUT)
            nc.gpsimd.dma_gather(
                g2[:], yv, i16g2[:, piece * PIECE_C * 8:(piece + 1) * PIECE_C * 8],
                P * PIECE_C, P * PIECE_C, 2 * COUT, elem_step=COUT)
            for j in range(PIECE_C):
                c = piece * PIECE_C + j
                v = chain.tile([P, COUT], BF16, tag="cv")
                t2 = chain.tile([P, COUT], BF16, tag="ct2")
                nc.vector.tensor_tensor(
                    v[:], g1[:, j, 0:COUT],
                    wq["w00"][:, c:c + 1].to_broadcast([P, COUT]), op=ALU.mult)
                nc.vector.tensor_tensor(
                    t2[:], g1[:, j, COUT:2 * COUT],
                    wq["w01"][:, c:c + 1].to_broadcast([P, COUT]), op=ALU.mult)
                nc.vector.tensor_tensor(v[:], v[:], t2[:], op=ALU.add)
                nc.vector.tensor_tensor(
                    t2[:], g2[:, j, 0:COUT],
                    wq["w10"][:, c:c + 1].to_broadcast([P, COUT]), op=ALU.mult)
                nc.vector.tensor_tensor(v[:], v[:], t2[:], op=ALU.add)
                nc.vector.tensor_tensor(
                    t2[:], g2[:, j, COUT:2 * COUT],
                    wq["w11"][:, c:c + 1].to_broadcast([P, COUT]), op=ALU.mult)
                nc.vector.tensor_tensor(v[:], v[:], t2[:], op=ALU.add)
                pay = chain.tile([P, TW], F32, tag="cpay")
                nc.vector.memset(pay[:, 257:TW], 0.0)
                nc.vector.tensor_copy(pay[:, 0:COUT], v[:])
                nc.vector.tensor_copy(pay[:, COUT:COUT + 1], aggw[:, c:c + 1])
                tp = psum.tile([P, P], F32, tag="tpp")
                nc.tensor.transpose(
                    tp[:], in_=iagf[:, c:c + 1].to_broadcast([P, P]),
                    identity=ident[:])
                tw_ = chain.tile([P, P], F32, tag="ctw")
                nc.vector.tensor_copy(tw_[:], tp[:])
                s = chain.tile([P, P], F32, tag="csel")
                nc.vector.tensor_tensor(
                    s[:], iagf[:, c:c + 1].to_broadcast([P, P]), tw_[:],
                    op=ALU.is_equal)
                cp = psum.tile([P, TW], F32, tag="cpp")
                nc.tensor.matmul(cp[:], lhsT=s[:], rhs=pay[:], start=True, stop=True)
                paysc = chain.tile([P, 1, TW], F32, tag="cpaysc")
                nc.scalar.copy(paysc[:, 0, :], cp[:])
                nc.gpsimd.dma_scatter_add(
                    tok_tbl[:], paysc[:], i16C[:, c * 8:(c + 1) * 8],
                    P, P, TW, elem_step=TW)

    if DEBUG:
        with tc.tile_pool(name="dbgt", bufs=2) as dp:
            for t in range(N // P):
                r_ = dp.tile([P, TW], F32, tag="dbgt")
                nc.sync.dma_start(r_[:], tok_tbl[t * P:(t + 1) * P, :])
                nc.sync.dma_start(T["dbgtok_t"].ap()[t * P:(t + 1) * P, :], r_[:])

    # ---------------- phase D ----------------
    if PHASES < 4:
        bulk.release(); persist.release(); psum1.release(); psum.release()
        cpool.release(); dram.release()
        return
    skw = cpool.tile([CIN, COUT], BF16)
    skf = cpool.tile([CIN, COUT], F32)
    nc.sync.dma_start(skf[:], skw_t.ap())
    nc.vector.tensor_copy(skw[:], skf[:])

    confst = persist.tile([P, N // P], F32)
    NT = N // P
    dstash = tc.alloc_tile_pool(name="dstash", bufs=1)
    stash = dstash.tile([P, N // P, COUT], BF16)
    with tc.tile_pool(name="dpool", bufs=4) as dpool:
        psS = psum1.tile([1, COUT], F32, tag="psS")
        psS2 = psum1.tile([1, COUT], F32, tag="psS2")
        for t in range(NT):
            tokrow = dpool.tile([P, TW], F32, tag="dtok")
            nc.sync.dma_start(tokrow[:], tok_tbl[t * P:(t + 1) * P, :])
            xc = dpool.tile([P, CIN], F32, tag="dxc")
            nc.sync.dma_start(xc[:], x_t.ap()[t * P:(t + 1) * P, :])
            xb = dpool.tile([P, CIN], BF16, tag="dxb")
            nc.vector.tensor_copy(xb[:], xc[:])
            tpx = psum.tile([P, P], BF16, tag="tpp")
            nc.tensor.transpose(tpx[:], in_=xb[:], identity=identb[:])
            xT = dpool.tile([P, P], BF16, tag="dxT")
            nc.vector.tensor_copy(xT[:], tpx[:])
            skp = psum.tile([P, COUT], F32, tag="cpp")
            nc.tensor.matmul(skp[:], lhsT=xT[:], rhs=skw[:], start=True, stop=True)
            rd = dpool.tile([P, 1], F32, tag="drd")
            nc.vector.tensor_scalar(rd[:], tokrow[:, COUT:COUT + 1], EPS, None,
                                    op0=ALU.add)
            nc.vector.reciprocal(rd[:], rd[:])
            tokc = dpool.tile([P, COUT], F32, tag="dtokc")
            nc.vector.scalar_tensor_tensor(tokc[:], tokrow[:, 0:COUT], rd[:], skp[:],
                                           op0=ALU.mult, op1=ALU.add)
            nc.vector.tensor_copy(stash[:, t, :], tokc[:])
            nc.tensor.matmul(psS[:], lhsT=ones_col[:], rhs=tokc[:],
                             start=(t == 0), stop=(t == NT - 1))
            sq = dpool.tile([P, COUT], F32, tag="dsq")
            nc.scalar.activation(sq[:], tokc[:], ACTF.Square)
            nc.tensor.matmul(psS2[:], lhsT=ones_col[:], rhs=sq[:],
                             start=(t == 0), stop=(t == NT - 1))

        stat = cpool.tile([1, 2 * COUT], F32)
        nc.vector.tensor_copy(stat[:, 0:COUT], psS[:])
        nc.vector.tensor_copy(stat[:, COUT:], psS2[:])
        nc.sync.dma_start(bncA[:], stat[:])
        nc.gpsimd.collective_compute(
            "AllReduce", ALU.add,
            replica_groups=[[0, 1, 2, 3], [4, 5, 6, 7]],
            ins=[bncA.opt()], outs=[bncB.opt()])
        nc.sync.dma_start(stat[:], bncB[:])

        gam = cpool.tile([1, COUT], F32)
        bet = cpool.tile([1, COUT], F32)
        cfw1 = cpool.tile([1, COUT], F32)
        cfb1 = cpool.tile([1, 1], F32)
        nc.sync.dma_start(gam[:], bng_t.ap())
        nc.sync.dma_start(bet[:], bnb_t.ap())
        nc.sync.dma_start(cfw1[:], cfw_t.ap())
        nc.sync.dma_start(cfb1[:], cfb_t.ap())
        mu = cpool.tile([1, COUT], F32)
        nc.vector.tensor_scalar(mu[:], stat[:, 0:COUT], 1.0 / (4 * N), None,
                                op0=ALU.mult)
        var = cpool.tile([1, COUT], F32)
        nc.vector.tensor_scalar(var[:], stat[:, COUT:], 1.0 / (4 * N), None,
                                op0=ALU.mult)
        musq = cpool.tile([1, COUT], F32)
        nc.vector.tensor_tensor(musq[:], mu[:], mu[:], op=ALU.mult)
        nc.vector.tensor_tensor(var[:], var[:], musq[:], op=ALU.subtract)
        nc.vector.tensor_scalar(var[:], var[:], BN_EPS, None, op0=ALU.add)
        rstd = cpool.tile([1, COUT], F32)
        nc.scalar.activation(rstd[:], var[:], ACTF.Sqrt)
        nc.vector.reciprocal(rstd[:], rstd[:])
        scl1 = cpool.tile([1, COUT], F32)
        nc.vector.tensor_tensor(scl1[:], gam[:], rstd[:], op=ALU.mult)
        sh1 = cpool.tile([1, COUT], F32)
        nc.vector.tensor_tensor(sh1[:], mu[:], scl1[:], op=ALU.mult)
        nc.vector.tensor_tensor(sh1[:], bet[:], sh1[:], op=ALU.subtract)
        scl = cpool.tile([P, COUT], F32)
        shf = cpool.tile([P, COUT], F32)
        cfwb = cpool.tile([P, COUT], F32)
        cfbb = cpool.tile([P, 1], F32)
        nc.gpsimd.partition_broadcast(scl[:], scl1[:], channels=P)
        nc.gpsimd.partition_broadcast(shf[:], sh1[:], channels=P)
        nc.gpsimd.partition_broadcast(cfwb[:], cfw1[:], channels=P)
        nc.gpsimd.partition_broadcast(cfbb[:], cfb1[:], channels=P)

        for t in range(NT):
            tokn = dpool.tile([P, COUT], F32, tag="dtokn")
            nc.vector.tensor_tensor(tokn[:], stash[:, t, :], scl[:], op=ALU.mult)
            nc.vector.tensor_tensor(tokn[:], tokn[:], shf[:], op=ALU.add)
            cf = dpool.tile([P, COUT], F32, tag="dcf")
            nc.vector.tensor_tensor(cf[:], tokn[:], cfwb[:], op=ALU.mult)
            cfs = dpool.tile([P, 1], F32, tag="dcfs")
            nc.vector.tensor_reduce(cfs[:], cf[:], axis=AX.X, op=ALU.add)
            nc.vector.tensor_tensor(cfs[:], cfs[:], cfbb[:], op=ALU.add)
            nc.vector.tensor_copy(confst[:, t:t + 1], cfs[:])
            xo = dpool.tile([P, COUT], F32, tag="dxo")
            nc.scalar.activation(xo[:], tokn[:], ACTF.Relu)
            nc.sync.dma_start(xout_t.ap()[t * P:(t + 1) * P, :], xo[:])
            trow = dpool.tile([P, 384], BF16, tag="dtrow")
            nc.vector.tensor_copy(trow[:, 0:COUT], tokn[:])
            chi = dpool.tile([P, 1], BF16, tag="dchi")
            nc.vector.tensor_copy(chi[:], cfs[:])
            nc.vector.tensor_copy(trow[:, COUT:COUT + 1], chi[:])
            chif = dpool.tile([P, 1], F32, tag="dchif")
            nc.vector.tensor_copy(chif[:], chi[:])
            nc.vector.tensor_tensor(chif[:], cfs[:], chif[:], op=ALU.subtract)
            nc.vector.tensor_copy(trow[:, COUT + 1:COUT + 2], chif[:])
            nc.vector.memset(trow[:, COUT + 2:384], 0.0)
            nc.sync.dma_start(tokn_tbl[t * P:(t + 1) * P, :], trow[:])
        tpc = psum.tile([P, P], F32, tag="tpp")
        nc.tensor.transpose(tpc[:], in_=confst[:], identity=ident[:])
        confT = dpool.tile([P, P], F32, tag="dconfT")
        nc.vector.tensor_copy(confT[:], tpc[:])
        nc.sync.dma_start(
            dataclasses.replace(conf_t.ap(), ap=[[P, P], [1, P]]), confT[:])

    dstash.release()

    # ---------------- phase E ----------------
    if PHASES < 5:
        bulk.release(); persist.release(); psum1.release(); psum.release()
        cpool.release(); dram.release()
        return
    wptst = persist.tile([P, NCOL], F32)
    PIECE_E = 4
    with tc.tile_pool(name="epool", bufs=2) as epool, \
         tc.tile_pool(name="echain", bufs=4) as chain, \
         tc.tile_pool(name="eidx", bufs=1) as eidx:
        i16x2 = load_idx(eidx, 0)
        i16E = load_idx(eidx, 4)
        for piece in range(NCOL // PIECE_E):
            g = epool.tile([P, PIECE_E, 384], BF16, tag="egbuf")
            nc.gpsimd.dma_gather(
                g[:], tokn_tbl[:],
                i16x2[:, piece * PIECE_E * 8:(piece + 1) * PIECE_E * 8],
                P * PIECE_E, P * PIECE_E, 384)
            for j in range(PIECE_E):
                c = piece * PIECE_E + j
                cfp = chain.tile([P, 2], F32, tag="ecfp")
                nc.vector.tensor_copy(cfp[:], g[:, j, COUT:COUT + 2])
                cfs = chain.tile([P, 1], F32, tag="ecfs")
                nc.vector.tensor_tensor(cfs[:], cfp[:, 0:1], cfp[:, 1:2], op=ALU.add)
                wpt = chain.tile([P, 1], F32, tag="ewpt")
                nc.scalar.activation(wpt[:], cfs[:], ACTF.Exp)
                nc.vector.tensor_copy(wptst[:, c:c + 1], wpt[:])
                pay = chain.tile([P, TW], F32, tag="epay")
                nc.vector.memset(pay[:, 257:TW], 0.0)
                nc.vector.tensor_tensor(pay[:, 0:COUT], g[:, j, 0:COUT],
                                        wpt[:].to_broadcast([P, COUT]), op=ALU.mult)
                nc.vector.tensor_copy(pay[:, COUT:COUT + 1], wpt[:])
                tp = psum.tile([P, P], F32, tag="tpp")
                nc.tensor.transpose(
                    tp[:], in_=cell64f[:, c:c + 1].to_broadcast([P, P]),
                    identity=ident[:])
                tw_ = chain.tile([P, P], F32, tag="etw")
                nc.vector.tensor_copy(tw_[:], tp[:])
                s = chain.tile([P, P], F32, tag="esel")
                nc.vector.tensor_tensor(
                    s[:], cell64f[:, c:c + 1].to_broadcast([P, P]), tw_[:],
                    op=ALU.is_equal)
                cp = psum.tile([P, TW], F32, tag="cpp")
                nc.tensor.matmul(cp[:], lhsT=s[:], rhs=pay[:], start=True, stop=True)
                paysc = chain.tile([P, 1, TW], F32, tag="epaysc")
                nc.scalar.copy(paysc[:, 0, :], cp[:])
                nc.gpsimd.dma_scatter_add(
                    clu_tbl[:], paysc[:], i16E[:, c * 8:(c + 1) * 8],
                    P, P, TW, elem_step=TW)

    # x_down + recip64 + agg_weight_down
    with tc.tile_pool(name="fpool", bufs=2) as fpool, \
         tc.tile_pool(name="fidx", bufs=1) as fidx:
        recst = persist.tile([P, NC64 // P], F32)
        for t in range(NC64 // P):
            row = fpool.tile([P, TW], F32, tag="frow")
            nc.sync.dma_start(row[:], clu_tbl[t * P:(t + 1) * P, :])
            rd = fpool.tile([P, 1], F32, tag="frd")
            nc.vector.tensor_scalar(rd[:], row[:, COUT:COUT + 1], EPS, None,
                                    op0=ALU.add)
            nc.vector.reciprocal(rd[:], rd[:])
            nc.vector.tensor_copy(recst[:, t:t + 1], rd[:])
            xd = fpool.tile([P, COUT], F32, tag="fxd")
            nc.vector.scalar_tensor_tensor(xd[:], row[:, 0:COUT], rd[:],
                                           row[:, 0:COUT],
                                           op0=ALU.mult, op1=ALU.bypass)
            nc.scalar.activation(xd[:], xd[:], ACTF.Relu)
            nc.sync.dma_start(xdown_t.ap()[t * P:(t + 1) * P, :], xd[:])
        rsv = recst[:]
        nc.sync.dma_start(
            dataclasses.replace(
                rec64_tbl[:],
                ap=[[64, P], [64 * P, NC64 // P], [1, 1]]),
            dataclasses.replace(rsv, ap=[rsv.ap[0], rsv.ap[1], [1, 1]]))

        i16r = load_idx(fidx, 7)
        PIECE_R = 8
        for piece in range(NCOL // PIECE_R):
            g = fpool.tile([P, PIECE_R, 64], F32, tag="frg")
            nc.gpsimd.dma_gather(
                g[:], rec64_tbl[:],
                i16r[:, piece * PIECE_R * 8:(piece + 1) * PIECE_R * 8],
                P * PIECE_R, P * PIECE_R, 64)
            w_ = fpool.tile([P, PIECE_R], F32, tag="fwq")
            nc.vector.tensor_tensor(
                w_[:], aggw[:, piece * PIECE_R:(piece + 1) * PIECE_R],
                wptst[:, piece * PIECE_R:(piece + 1) * PIECE_R], op=ALU.mult)
            nc.vector.tensor_tensor(w_[:], w_[:], g[:, :, 0], op=ALU.mult)
            nc.vector.tensor_copy(wptst[:, piece * PIECE_R:(piece + 1) * PIECE_R],
                                  w_[:])
        mx = fpool.tile([P, 1], F32, tag="fmx")
        nc.vector.tensor_reduce(mx[:], wptst[:], axis=AX.X, op=ALU.max)
        mxr = fpool.tile([P, 1], F32, tag="fmxr")
        nc.gpsimd.partition_all_reduce(mxr[:], mx[:], P, bass_isa.ReduceOp.max)
        nc.vector.reciprocal(mxr[:], mxr[:])
        awd = fpool.tile([P, NCOL], F32, tag="fawd")
        nc.vector.tensor_tensor(awd[:], wptst[:], mxr[:].to_broadcast([P, NCOL]),
                                op=ALU.mult)
        nc.sync.dma_start(awd_t.ap().rearrange("(p c) d -> p (c d)", p=P), awd[:])

    bulk.release()
    persist.release()
    psum1.release()
    psum.release()
    cpool.release()
    dram.release()


def _build():
    nc = bacc.Bacc("TRN2", target_bir_lowering=False, debug=False, num_devices=8)
    T = {}
    T["x_t"] = nc.dram_tensor("x", [N, CIN], F32, kind="ExternalInput")
    T["loc_t"] = nc.dram_tensor("loc", [N0, 2], F32, kind="ExternalInput")
    T["ia_t"] = nc.dram_tensor("idx_agg", [N0], I32, kind="ExternalInput")
    T["aw_t"] = nc.dram_tensor("agg_w", [N0], F32, kind="ExternalInput")
    T["cw_t"] = nc.dram_tensor("conv_w", [9 * CIN, COUT], F32, kind="ExternalInput")
    T["cb_t"] = nc.dram_tensor("conv_b", [1, COUT], F32, kind="ExternalInput")
    T["skw_t"] = nc.dram_tensor("skip_wT", [CIN, COUT], F32, kind="ExternalInput")
    T["bng_t"] = nc.dram_tensor("bn_g", [1, COUT], F32, kind="ExternalInput")
    T["bnb_t"] = nc.dram_tensor("bn_b", [1, COUT], F32, kind="ExternalInput")
    T["cfw_t"] = nc.dram_tensor("conf_w", [1, COUT], F32, kind="ExternalInput")
    T["cfb_t"] = nc.dram_tensor("conf_b", [1, 1], F32, kind="ExternalInput")
    T["xdown_t"] = nc.dram_tensor("x_down", [NC64, COUT], F32, kind="ExternalOutput")
    T["xout_t"] = nc.dram_tensor("x_out", [N, COUT], F32, kind="ExternalOutput")
    T["conf_t"] = nc.dram_tensor("conf", [N, 1], F32, kind="ExternalOutput")
    T["awd_t"] = nc.dram_tensor("agg_w_down", [N0, 1], F32, kind="ExternalOutput")
    T["iad_t"] = nc.dram_tensor("idx_agg_down", [N0], I32, kind="ExternalOutput")
    if DEBUG:
        T["dbgy_t"] = nc.dram_tensor("dbg_y", [HD * HD, COUT], F32, kind="ExternalOutput")
        T["dbgtok_t"] = nc.dram_tensor("dbg_tok", [N, TW], F32, kind="ExternalOutput")
        T["dbgmap_t"] = nc.dram_tensor("dbg_map", [2048, TW], F32, kind="ExternalOutput")
        T["dbg2_t"] = nc.dram_tensor("dbg2", [P, 4 * NCOL], F32, kind="ExternalOutput")
        T["dbg3_t"] = nc.dram_tensor("dbg3", [P, 2 * TW], I16, kind="ExternalOutput")
    with tile.TileContext(nc) as tc:
        _body(nc, tc, T)
    nc.finalize()
    return nc


def kernel(x, loc_orig, idx_agg, agg_weight, conv_w, conv_b, skip_w,
           bn_gamma, bn_beta, conf_w, conf_b, H=256, W=256):
    idx_in_dtype = np.asarray(idx_agg).dtype
    x = np.ascontiguousarray(np.asarray(x, np.float32))
    loc_orig = np.ascontiguousarray(np.asarray(loc_orig, np.float32))
    idx_agg_np = np.ascontiguousarray(np.asarray(idx_agg).astype(np.int32))
    agg_weight = np.ascontiguousarray(np.asarray(agg_weight, np.float32))
    B = x.shape[0]

    if "nc" not in _CACHE:
        _CACHE["nc"] = _build()
    nc = _CACHE["nc"]

    cwr = np.ascontiguousarray(
        np.asarray(conv_w, np.float32).reshape(9, CIN, COUT)).reshape(9 * CIN, COUT)
    skT = np.ascontiguousarray(np.asarray(skip_w, np.float32).T)
    in_maps = []
    for c in range(8):
        b = c % B
        in_maps.append({
            "x": x[b],
            "loc": loc_orig[b],
            "idx_agg": idx_agg_np[b],
            "agg_w": np.ascontiguousarray(agg_weight[b, :, 0]),
            "conv_w": cwr,
            "conv_b": np.asarray(conv_b, np.float32).reshape(1, COUT),
            "skip_wT": skT,
            "bn_g": np.asarray(bn_gamma, np.float32).reshape(1, COUT),
            "bn_b": np.asarray(bn_beta, np.float32).reshape(1, COUT),
            "conf_w": np.asarray(conf_w, np.float32).reshape(1, COUT),
            "conf_b": np.asarray(conf_b, np.float32).reshape(1, 1),
        })
    import time as _time
    _t0 = _time.time()
    res = bass_utils.run_bass_kernel_spmd(nc, in_maps, core_ids=list(range(8)))
    _CACHE["exec_wall_ns"] = int((_time.time() - _t0) * 1e9)
    r = res.results
    x_down = np.stack([r[b]["x_down"] for b in range(B)])
    x_out = np.stack([r[b]["x_out"] for b in range(B)])
    conf = np.stack([r[b]["conf"] for b in range(B)])
    awd = np.stack([r[b]["agg_w_down"] for b in range(B)])
    iad = np.stack([r[b]["idx_agg_down"] for b in range(B)])
    iad = iad.astype(idx_in_dtype if idx_in_dtype in (np.int32, np.int64)
                     else np.int32)
    return x_down, x_out, conf, awd, iad
